# revision 14
# baseline (speedup 1.0000x reference)
"""Trainium2 Bass kernel for nn_DecoderBlock (B=2,S=2048,D=1024,H=16,FF=4096).

Sharding (8 cores): core c -> batch b=c//4, head-group r=c%4 (heads 4r..4r+3).
- QKV projections column-sharded by heads; attention fully local per head group.
- Softmax mask folded into V (masked rows zeroed) + denominator as extra V
  column -> no mask/sum passes over the 2048x2048 score matrices.
- Wo row-sharded -> partial attn_out -> chunked ReduceScatter over each
  4-core group (bf16 wire) -> each core finishes LN1+FFN+LN2 for 512 tokens.
- All matmuls bf16 (fp32 PSUM accumulate). LN stats fp32 via bn_stats;
  rstd = exp(-0.5*ln(var+eps)) so Exp/Ln/Relu share ONE ACT table set.
"""
import math

import numpy as np
import ml_dtypes

import concourse.bass as bass
import concourse.mybir as mybir
import concourse.tile as tile
from concourse import bacc
from concourse import bass_utils
from concourse.hw_specs import get_activation_tables
from concourse.masks import make_identity

AF = mybir.ActivationFunctionType
OP = mybir.AlupOpType if hasattr(mybir, "AlupOpType") else mybir.AluOpType
BF16 = mybir.dt.bfloat16
F32 = mybir.dt.float32

B, S, D, H, FF = 2, 2048, 1024, 16, 4096
DH = D // H            # 64
HL = 4                 # local heads per core
DHL = HL * DH          # 256
P = 128
EPS = 1e-5
NQC = S // 512         # 4 query chunks
NCH = 2                # reduce-scatter chunks (1024 tokens each)
TOK = S // 4           # 512 output tokens per core

_orig_tables = get_activation_tables
_PATCHED = False


def _patch_act_tables():
    """Force Exp/Ln/Relu/Copy onto the single natural_log_exp_and_others set
    so no ACT table reloads (~2.7us each) happen mid-kernel."""
    global _PATCHED
    if _PATCHED:
        return
    strip = {AF.Exp, AF.Ln, AF.Relu, AF.Copy, AF.Square, AF.Identity}

    def patched(arch):
        t = _orig_tables(arch)
        return {name: (fns if name == "natural_log_exp_and_others" else fns - strip)
                for name, fns in t.items()}

    bacc.get_activation_tables = patched
    _PATCHED = True


def _build(nkb):
    """Build + compile the SPMD program. nkb = number of valid 128-key blocks
    (= ceil(max(valid_lens)/128)); key blocks >= nkb are fully masked and
    skipped (mask still applied via V', so smaller-vl batches stay correct)."""
    _patch_act_tables()
    nc = bacc.Bacc("TRN2", target_bir_lowering=False, debug=False,
                   enable_asserts=False, num_devices=8)

    q_bf = nc.dram_tensor("q_bf", [S, D], BF16, kind="ExternalInput").ap()
    k_bf = nc.dram_tensor("k_bf", [S, D], BF16, kind="ExternalInput").ap()
    v_bf = nc.dram_tensor("v_bf", [S, D], BF16, kind="ExternalInput").ap()
    wq_d = nc.dram_tensor("wq", [D, DHL], BF16, kind="ExternalInput").ap()
    wk_d = nc.dram_tensor("wk", [D, DHL], BF16, kind="ExternalInput").ap()
    wv_d = nc.dram_tensor("wv", [D, DHL], BF16, kind="ExternalInput").ap()
    wo_d = nc.dram_tensor("wo", [DHL, D], BF16, kind="ExternalInput").ap()
    w1_d = nc.dram_tensor("w1", [D, FF], BF16, kind="ExternalInput").ap()
    w2_d = nc.dram_tensor("w2", [FF, D], BF16, kind="ExternalInput").ap()
    b1_d = nc.dram_tensor("b1f", [FF], F32, kind="ExternalInput").ap()
    b2_d = nc.dram_tensor("b2b", [D], BF16, kind="ExternalInput").ap()
    lnp_d = nc.dram_tensor("lnp", [4, D], BF16, kind="ExternalInput").ap()
    mask_d = nc.dram_tensor("maskf", [S], F32, kind="ExternalInput").ap()
    qres_d = nc.dram_tensor("qres", [TOK, D], BF16, kind="ExternalInput").ap()
    out_d = nc.dram_tensor("out", [TOK, D], F32, kind="ExternalOutput").ap()

    n_ksc = (nkb + 3) // 4    # 512-row source chunks needed for K/V proj

    def bcast(ap, n_part):
        """partition-broadcast view of a DRAM AP (step-0 partition dim)."""
        return bass.AP(tensor=ap.tensor, offset=ap.offset,
                       ap=[[0, n_part]] + [list(x) for x in ap.ap])

    from contextlib import ExitStack
    with tile.TileContext(nc) as tc:
        with ExitStack() as _es:
            def _pool(name, bufs, space="SBUF"):
                return _es.enter_context(
                    tc.tile_pool(name=name, bufs=bufs, space=space))

            singles = _pool("singles", 1)   # constants + big resident tensors
            xtp = _pool("xtp", 2)           # transposed src chunks
            qtp = _pool("qtp", 2)           # QT per chunk
            ptp = _pool("ptp", 2)           # P^T stripes
            ctp = _pool("ctp", 2)           # ctxT per chunk
            smallp = _pool("smallp", 8)     # tiny vectors
            recipp = _pool("recipp", 2)     # recip rows
            lntp = _pool("lntp", 1)         # LN temp
            rbp = _pool("rbp", 2)           # recip broadcast
            tmbp = _pool("tmbp", 1)         # head-B shift temp
            aop = _pool("aop", 2)           # attn-out staging
            w1p = _pool("w1p", 3)           # streamed w1 tiles
            ffp = _pool("ffp", 2)           # ffn misc tiles
            ytp = _pool("ytp", 1)           # Y^T
            h1p = _pool("h1p", 1)           # H1^T
            psS = _pool("psS", 1, "PSUM")
            psCtx = _pool("psCtx", 2, "PSUM")
            psMM = _pool("psMM", 2, "PSUM")
            psH1 = _pool("psH1", 2, "PSUM")
            dramp = _pool("dramp", 2, "DRAM")
            # ---------- constants ----------
            ident = singles.tile([P, P], BF16)
            make_identity(nc, ident)
            eps_sb = singles.tile([P, 1], F32)
            nc.vector.memset(eps_sb, EPS)
            mask_sb = singles.tile([P, S // P], F32)
            nc.sync.dma_start(mask_sb, mask_d.rearrange("(a p) -> p a", p=P))
            b1_sb = singles.tile([P, FF // P], F32)
            nc.sync.dma_start(b1_sb, b1_d.rearrange("(a p) -> p a", p=P))
            b2rep = singles.tile([P, D], BF16)
            nc.sync.dma_start(b2rep, bcast(b2_d, P))
            lnp_sb = singles.tile([P, 4, D], BF16)
            nc.sync.dma_start(lnp_sb, bcast(lnp_d, P))

            wq_sb = singles.tile([P, 8, DHL], BF16)
            nc.sync.dma_start(wq_sb, wq_d.rearrange("(a p) n -> p a n", p=P))
            wk_sb = singles.tile([P, 8, DHL], BF16)
            nc.sync.dma_start(wk_sb, wk_d.rearrange("(a p) n -> p a n", p=P))
            wv_sb = singles.tile([P, 8, DHL], BF16)
            nc.sync.dma_start(wv_sb, wv_d.rearrange("(a p) n -> p a n", p=P))
            wo_sb = singles.tile([P, 2, D], BF16)
            nc.sync.dma_start(wo_sb, wo_d.rearrange("(a p) n -> p a n", p=P))
            w2_sb = singles.tile([P, FF // P, D], BF16)
            nc.sync.dma_start(w2_sb, w2_d.rearrange("(a p) n -> p a n", p=P))

            # ---------- K^T projection ----------
            KT = singles.tile([P, 2, nkb * P], BF16)
            for sc in range(n_ksc):
                kTc = xtp.tile([P, 8, 512], BF16, tag="xT")
                nc.sync.dma_start_transpose(kTc, k_bf[sc * 512:(sc + 1) * 512, :])
                w = min(512, nkb * P - sc * 512)
                for ob in range(2):
                    ps = psMM.tile([P, 512], F32, tag="mm")
                    for ib in range(8):
                        nc.tensor.matmul(ps[:, :w], wk_sb[:, ib, ob * P:(ob + 1) * P],
                                         kTc[:, ib, :w], start=(ib == 0), stop=(ib == 7))
                    nc.vector.tensor_copy(KT[:, ob, sc * 512:sc * 512 + w], ps[:, :w])

            # ---------- V projection + mask + denom column ----------
            Vp = singles.tile([P, nkb, HL * 65], BF16)
            for sc in range(n_ksc):
                vTc = xtp.tile([P, 8, 512], BF16, tag="xT")
                nc.sync.dma_start_transpose(vTc, v_bf[sc * 512:(sc + 1) * 512, :])
                for s4 in range(4):
                    kb = sc * 4 + s4
                    if kb >= nkb:
                        break
                    ps = psMM.tile([P, 512], F32, tag="mm")
                    for ib in range(8):
                        nc.tensor.matmul(ps[:, :DHL], vTc[:, ib, s4 * P:(s4 + 1) * P],
                                         wv_sb[:, ib, :], start=(ib == 0), stop=(ib == 7))
                    vsl = Vp[:, kb, :].rearrange("p (h e) -> p h e", h=HL)
                    m1 = mask_sb[:, kb:kb + 1, None]
                    nc.vector.tensor_tensor(
                        vsl[:, :, 0:64],
                        ps[:, :DHL].rearrange("p (h e) -> p h e", e=64),
                        m1.to_broadcast([P, HL, 64]), OP.mult)
                    nc.vector.tensor_copy(vsl[:, :, 64:65],
                                          m1.to_broadcast([P, HL, 1]))

            # ---------- helpers for FFN phase ----------
            def ffn_chunk(ch, rs_out):
                ybfs = []
                yT = ytp.tile([P, 8, 256], BF16, tag="yT")
                for sblk in range(2):
                    xsb = ffp.tile([P, D], BF16, tag="rsx")
                    nc.sync.dma_start(
                        xsb, rs_out.rearrange("(t p) d -> p t d", p=P)[:, sblk, :])
                    qrs = ffp.tile([P, D], BF16, tag="qrs")
                    nc.sync.dma_start(
                        qrs, qres_d[ch * 256 + sblk * P: ch * 256 + (sblk + 1) * P, :])
                    xx = ffp.tile([P, D], BF16, tag="xx")
                    nc.vector.tensor_tensor(xx, xsb, qrs, OP.add)
                    ybf = ffp.tile([P, D], BF16, tag="ybf")
                    _layernorm(xx, 0, ybf, F32_out=None)
                    ybfs.append(ybf)
                    for db in range(8):
                        tp = psMM.tile([P, P], BF16, tag="mm")
                        nc.tensor.transpose(tp, ybf[:, db * P:(db + 1) * P], ident)
                        nc.vector.tensor_copy(yT[:, db, sblk * P:(sblk + 1) * P], tp)
                h1T = h1p.tile([P, FF // P, 256], BF16, tag="h1T")
                for fb in range(FF // P):
                    w1t = w1p.tile([P, 8, P], BF16, tag="w1t")
                    nc.sync.dma_start(
                        w1t, w1_d.rearrange("(a p) f -> p a f", p=P)[:, :, fb * P:(fb + 1) * P])
                    hps = psH1.tile([P, 256], F32, tag="h1")
                    for db in range(8):
                        nc.tensor.matmul(hps, w1t[:, db, :], yT[:, db, :],
                                         start=(db == 0), stop=(db == 7))
                    nc.scalar.activation(h1T[:, fb, :], hps, AF.Relu,
                                         bias=b1_sb[:, fb:fb + 1])
                for sblk in range(2):
                    fy = ffp.tile([P, D], BF16, tag="xx")
                    for dc in range(2):
                        fps = psMM.tile([P, 512], F32, tag="mm")
                        for fb in range(FF // P):
                            nc.tensor.matmul(fps, h1T[:, fb, sblk * P:(sblk + 1) * P],
                                             w2_sb[:, fb, dc * 512:(dc + 1) * 512],
                                             start=(fb == 0), stop=(fb == FF // P - 1))
                        nc.vector.tensor_tensor(fy[:, dc * 512:(dc + 1) * 512], fps,
                                                b2rep[:, dc * 512:(dc + 1) * 512], OP.add)
                    nc.vector.tensor_tensor(fy, fy, ybfs[sblk], OP.add)
                    ost = ffp.tile([P, D], F32, tag="ost")
                    _layernorm(fy, 2, None, F32_out=ost)
                    nc.sync.dma_start(
                        out_d[ch * 256 + sblk * P: ch * 256 + (sblk + 1) * P, :], ost)

            def _layernorm(xx, gidx, bf16_out, F32_out):
                """LN over free dim D of xx [P, D]; gain=lnp[gidx], bias=lnp[gidx+1]."""
                stats = smallp.tile([P, 2, 6], F32, tag="stats")
                for h in range(2):
                    nc.vector.bn_stats(stats[:, h, :], xx[:, h * 512:(h + 1) * 512])
                mv = smallp.tile([P, 2], F32, tag="mv")
                nc.vector.bn_aggr(mv, stats)
                lnv = smallp.tile([P, 1], F32, tag="lnv")
                nc.scalar.activation(lnv, mv[:, 1:2], AF.Ln, bias=eps_sb)
                rstd = smallp.tile([P, 1], F32, tag="rstd")
                nc.scalar.activation(rstd, lnv, AF.Exp, scale=-0.5)
                t = lntp.tile([P, D], F32, tag="lnt")
                nc.vector.tensor_scalar(t, xx, mv[:, 0:1], rstd,
                                        OP.subtract, OP.mult)
                g = lnp_sb[:, gidx, :]
                bvec = lnp_sb[:, gidx + 1, :]
                if bf16_out is not None:
                    nc.vector.tensor_tensor(t, t, g, OP.mult)
                    nc.vector.tensor_tensor(bf16_out, t, bvec, OP.add)
                else:
                    nc.vector.tensor_tensor(t, t, g, OP.mult)
                    nc.vector.tensor_tensor(F32_out, t, bvec, OP.add)

            # ---------- main attention loop ----------
            cin = [None, None]
            rs_out = [None, None]
            for ch in range(NCH):
                cin[ch] = dramp.tile([1024, D], BF16, tag="cin", name=f"cin{ch}")
                rs_out[ch] = dramp.tile([256, D], BF16, tag="rsout", name=f"rsout{ch}")

            for qc in range(NQC):
                ch = qc // 2
                # Q^T projection for this 512-query chunk
                qTc = xtp.tile([P, 8, 512], BF16, tag="xT")
                nc.sync.dma_start_transpose(qTc, q_bf[qc * 512:(qc + 1) * 512, :])
                QT = qtp.tile([P, 2, 512], BF16, tag="QT")
                for ob in range(2):
                    ps = psMM.tile([P, 512], F32, tag="mm")
                    for ib in range(8):
                        nc.tensor.matmul(ps, wq_sb[:, ib, ob * P:(ob + 1) * P],
                                         qTc[:, ib, :], start=(ib == 0), stop=(ib == 7))
                    nc.vector.tensor_copy(QT[:, ob, :], ps)

                ctxT_sb = ctp.tile([P, 2, 512], BF16, tag="ctxT")
                for hp in range(2):
                    ctxA = psCtx.tile([65, 512], F32, tag="ctx")
                    ctxB = psCtx.tile([65, 512], F32, tag="ctx")
                    for kb in range(nkb):
                        st = psS.tile([P, 1024], F32, tag="st")
                        nc.tensor.matmul(st[:, 0:512],
                                         KT[0:64, hp, kb * P:(kb + 1) * P],
                                         QT[0:64, hp, :],
                                         tile_position=(0, 0), start=True, stop=True)
                        nc.tensor.matmul(st[:, 512:1024],
                                         KT[64:128, hp, kb * P:(kb + 1) * P],
                                         QT[64:128, hp, :],
                                         tile_position=(64, 0), start=True, stop=True)
                        Pt = ptp.tile([P, 1024], BF16, tag="Pt")
                        nc.scalar.activation(Pt, st, AF.Exp, scale=0.125)
                        vsl = Vp[:, kb, :].rearrange("p (h e) -> p h e", h=HL)
                        nc.tensor.matmul(ctxA, vsl[:, 2 * hp, :], Pt[:, 0:512],
                                         start=(kb == 0), stop=(kb == nkb - 1))
                        nc.tensor.matmul(ctxB, vsl[:, 2 * hp + 1, :], Pt[:, 512:1024],
                                         start=(kb == 0), stop=(kb == nkb - 1))
                    # divide by denominators (row 64), assemble ctxT block hp
                    rA = recipp.tile([1, 512], F32, tag="recip")
                    nc.vector.reciprocal(rA, ctxA[64:65, :])
                    rB = recipp.tile([1, 512], F32, tag="recip")
                    nc.vector.reciprocal(rB, ctxB[64:65, :])
                    # partition-broadcast via DRAM bounce (SBUF src can't step-0)
                    rd = dramp.tile([2, 512], F32, tag="rd", name=f"rd_{qc}_{hp}")
                    nc.sync.dma_start(rd[0:1, :], rA)
                    nc.sync.dma_start(rd[1:2, :], rB)
                    rbA = rbp.tile([64, 512], F32, tag="rb")
                    nc.sync.dma_start(rbA, bcast(rd[0, :], 64))
                    rbB = rbp.tile([64, 512], F32, tag="rb")
                    nc.sync.dma_start(rbB, bcast(rd[1, :], 64))
                    nc.vector.tensor_tensor(ctxT_sb[0:64, hp, :], ctxA[0:64, :],
                                            rbA, OP.mult)
                    tmpB = tmbp.tile([64, 512], BF16, tag="tmpB")
                    nc.vector.tensor_tensor(tmpB, ctxB[0:64, :], rbB, OP.mult)
                    nc.sync.dma_start(ctxT_sb[64:128, hp, :], tmpB)

                # Wo partial for this chunk -> collective input
                for sblk in range(4):
                    ao = aop.tile([P, D], BF16, tag="ao")
                    for dc in range(2):
                        ps = psMM.tile([P, 512], F32, tag="mm")
                        for db in range(2):
                            nc.tensor.matmul(ps, ctxT_sb[:, db, sblk * P:(sblk + 1) * P],
                                             wo_sb[:, db, dc * 512:(dc + 1) * 512],
                                             start=(db == 0), stop=(db == 1))
                        nc.vector.tensor_copy(ao[:, dc * 512:(dc + 1) * 512], ps)
                    nc.sync.dma_start(
                        cin[ch].rearrange("(t p) d -> p t d", p=P)[:, (qc % 2) * 4 + sblk, :],
                        ao)

                if qc == 1:
                    nc.gpsimd.collective_compute(
                        "ReduceScatter", OP.add,
                        replica_groups=[[0, 1, 2, 3], [4, 5, 6, 7]],
                        ins=[cin[0].opt()], outs=[rs_out[0].opt()])
                if qc == 2:
                    ffn_chunk(0, rs_out[0])
                if qc == 3:
                    nc.gpsimd.collective_compute(
                        "ReduceScatter", OP.add,
                        replica_groups=[[0, 1, 2, 3], [4, 5, 6, 7]],
                        ins=[cin[1].opt()], outs=[rs_out[1].opt()])
                    ffn_chunk(1, rs_out[1])

    nc.compile()
    return nc


_CACHE = {}


def _get_nc(nkb):
    if nkb not in _CACHE:
        _CACHE[nkb] = _build(nkb)
    return _CACHE[nkb]


LAST_RESULT = None
LAST_CTX = None


def kernel(q, k, v, Wq, Wk, Wv, Wo, w1, b1, w2, b2,
           ln1_g, ln1_b, ln2_g, ln2_b, valid_lens, _trace=False):
    global LAST_RESULT
    bf = ml_dtypes.bfloat16
    q = np.asarray(q, np.float32); k = np.asarray(k, np.float32)
    v = np.asarray(v, np.float32)
    vl = np.asarray(valid_lens).astype(np.int64)
    nkb = int(min(S // P, max(1, math.ceil(float(vl.max()) / P))))
    nc = _get_nc(nkb)

    w1b = np.ascontiguousarray(np.asarray(w1, np.float32)).astype(bf)
    w2b = np.ascontiguousarray(np.asarray(w2, np.float32)).astype(bf)
    lnp = np.stack([np.asarray(x, np.float32) for x in (ln1_g, ln1_b, ln2_g, ln2_b)]
                   ).astype(bf)
    b1f = np.asarray(b1, np.float32)
    b2b = np.asarray(b2, np.float32).astype(bf)

    in_maps = []
    tok_idx_all = []
    for c in range(8):
        b = c // 4
        r = c % 4
        cols = slice(r * DHL, (r + 1) * DHL)
        mask = (np.arange(S) < int(vl[b])).astype(np.float32)
        tok_idx = np.concatenate(
            [ch * 1024 + r * 256 + np.arange(256) for ch in range(NCH)])
        tok_idx_all.append(tok_idx)
        in_maps.append({
            "q_bf": q[b].astype(bf),
            "k_bf": k[b].astype(bf),
            "v_bf": v[b].astype(bf),
            "wq": np.ascontiguousarray(np.asarray(Wq, np.float32)[:, cols]).astype(bf),
            "wk": np.ascontiguousarray(np.asarray(Wk, np.float32)[:, cols]).astype(bf),
            "wv": np.ascontiguousarray(np.asarray(Wv, np.float32)[:, cols]).astype(bf),
            "wo": np.ascontiguousarray(np.asarray(Wo, np.float32)[cols, :]).astype(bf),
            "w1": w1b, "w2": w2b, "b1f": b1f, "b2b": b2b, "lnp": lnp,
            "maskf": mask,
            "qres": np.ascontiguousarray(q[b][tok_idx]).astype(bf),
        })

    res = bass_utils.run_bass_kernel_spmd(nc, in_maps, core_ids=list(range(8)),
                                          trace=_trace)
    LAST_RESULT = res
    global LAST_CTX
    LAST_CTX = (nc, in_maps, nkb)

    out = np.empty((B, S, D), np.float32)
    for c in range(8):
        out[c // 4, tok_idx_all[c]] = res.results[c]["out"]
    return out


# revision 20
# speedup vs baseline: 1.1180x; 1.1180x over previous
"""Trainium2 Bass kernel for nn_DecoderBlock (B=2,S=2048,D=1024,H=16,FF=4096).

Sharding (8 cores): core c -> batch b=c//4, head-group r=c%4 (heads 4r..4r+3).
- QKV projections column-sharded by heads; attention fully local per head group.
- Softmax mask folded into V (masked rows zeroed) + denominator as extra V
  column -> no mask/sum passes over the 2048x2048 score matrices.
- Wo row-sharded -> partial attn_out -> chunked ReduceScatter over each
  4-core group (bf16 wire) -> each core finishes LN1+FFN+LN2 for 512 tokens.
- All matmuls bf16 (fp32 PSUM accumulate). LN stats fp32 via bn_stats;
  rstd = exp(-0.5*ln(var+eps)) so Exp/Ln/Relu share ONE ACT table set.
"""
import math

import numpy as np
import ml_dtypes

import concourse.bass as bass
import concourse.mybir as mybir
import concourse.tile as tile
from concourse import bacc
from concourse import bass_utils
from concourse.hw_specs import get_activation_tables
from concourse.masks import make_identity

AF = mybir.ActivationFunctionType
OP = mybir.AlupOpType if hasattr(mybir, "AlupOpType") else mybir.AluOpType
BF16 = mybir.dt.bfloat16
F32 = mybir.dt.float32

B, S, D, H, FF = 2, 2048, 1024, 16, 4096
DH = D // H            # 64
HL = 4                 # local heads per core
DHL = HL * DH          # 256
P = 128
EPS = 1e-5
NQC = S // 512         # 4 query chunks
NCH = 2                # reduce-scatter chunks (1024 tokens each)
TOK = S // 4           # 512 output tokens per core

_orig_tables = get_activation_tables
_PATCHED = False


def _patch_act_tables():
    """Force Exp/Ln/Relu/Copy onto the single natural_log_exp_and_others set
    so no ACT table reloads (~2.7us each) happen mid-kernel."""
    global _PATCHED
    if _PATCHED:
        return
    strip = {AF.Exp, AF.Ln, AF.Relu, AF.Copy, AF.Square, AF.Identity}

    def patched(arch):
        t = _orig_tables(arch)
        return {name: (fns if name == "natural_log_exp_and_others" else fns - strip)
                for name, fns in t.items()}

    bacc.get_activation_tables = patched
    _PATCHED = True


def _build(nkb):
    """Build + compile the SPMD program. nkb = number of valid 128-key blocks
    (= ceil(max(valid_lens)/128)); key blocks >= nkb are fully masked and
    skipped (mask still applied via V', so smaller-vl batches stay correct)."""
    _patch_act_tables()
    nc = bacc.Bacc("TRN2", target_bir_lowering=False, debug=False,
                   enable_asserts=False, num_devices=8)

    q_bf = nc.dram_tensor("q_bf", [S, D], BF16, kind="ExternalInput").ap()
    k_bf = nc.dram_tensor("k_bf", [S, D], BF16, kind="ExternalInput").ap()
    v_bf = nc.dram_tensor("v_bf", [S, D], BF16, kind="ExternalInput").ap()
    wq_d = nc.dram_tensor("wq", [D, DHL], BF16, kind="ExternalInput").ap()
    wk_d = nc.dram_tensor("wk", [D, DHL], BF16, kind="ExternalInput").ap()
    wv_d = nc.dram_tensor("wv", [D, DHL], BF16, kind="ExternalInput").ap()
    wo_d = nc.dram_tensor("wo", [DHL, D], BF16, kind="ExternalInput").ap()
    w1_d = nc.dram_tensor("w1", [D, FF], BF16, kind="ExternalInput").ap()
    w2_d = nc.dram_tensor("w2", [FF, D], BF16, kind="ExternalInput").ap()
    b1_d = nc.dram_tensor("b1f", [FF], F32, kind="ExternalInput").ap()
    b2_d = nc.dram_tensor("b2b", [D], BF16, kind="ExternalInput").ap()
    lnp_d = nc.dram_tensor("lnp", [4, D], BF16, kind="ExternalInput").ap()
    mask_d = nc.dram_tensor("maskf", [S], F32, kind="ExternalInput").ap()
    qres_d = nc.dram_tensor("qres", [TOK, D], BF16, kind="ExternalInput").ap()
    out_d = nc.dram_tensor("out", [TOK, D], F32, kind="ExternalOutput").ap()

    n_ksc = (nkb + 3) // 4    # 512-row source chunks needed for K/V proj

    def bcast(ap, n_part):
        """partition-broadcast view of a DRAM AP (step-0 partition dim)."""
        return bass.AP(tensor=ap.tensor, offset=ap.offset,
                       ap=[[0, n_part]] + [list(x) for x in ap.ap])

    from contextlib import ExitStack
    with tile.TileContext(nc) as tc:
        with ExitStack() as _es:
            def _pool(name, bufs, space="SBUF"):
                return _es.enter_context(
                    tc.tile_pool(name=name, bufs=bufs, space=space))

            singles = _pool("singles", 1)   # constants + big resident tensors
            xtp = _pool("xtp", 2)           # transposed src chunks
            qtp = _pool("qtp", 2)           # QT per chunk
            ptp = _pool("ptp", 2)           # P^T stripes
            ctp = _pool("ctp", 2)           # ctxT per chunk
            smallp = _pool("smallp", 8)     # tiny vectors
            recipp = _pool("recipp", 2)     # recip rows
            lntp = _pool("lntp", 1)         # LN temp
            rbp = _pool("rbp", 2)           # recip broadcast
            tmbp = _pool("tmbp", 1)         # head-B shift temp
            aop = _pool("aop", 2)           # attn-out staging
            w1p = _pool("w1p", 3)           # streamed w1 tiles
            ffp = _pool("ffp", 2)           # ffn misc tiles
            ytp = _pool("ytp", 1)           # Y^T
            h1p = _pool("h1p", 1)           # H1^T
            psS = _pool("psS", 2, "PSUM")
            psCtx = _pool("psCtx", 2, "PSUM")
            psMM = _pool("psMM", 2, "PSUM")
            dramp = _pool("dramp", 2, "DRAM")
            # ---------- constants ----------
            ident = singles.tile([P, P], BF16)
            make_identity(nc, ident)
            eps_sb = singles.tile([P, 1], F32)
            nc.vector.memset(eps_sb, EPS)
            mask_sb = singles.tile([P, S // P], F32)
            nc.sync.dma_start(mask_sb, mask_d.rearrange("(a p) -> p a", p=P))
            b1_sb = singles.tile([P, FF // P], F32)
            nc.sync.dma_start(b1_sb, b1_d.rearrange("(a p) -> p a", p=P))
            wq_sb = singles.tile([P, 8, DHL], BF16)
            nc.sync.dma_start(wq_sb, wq_d.rearrange("(a p) n -> p a n", p=P))
            wk_sb = singles.tile([P, 8, DHL], BF16)
            nc.sync.dma_start(wk_sb, wk_d.rearrange("(a p) n -> p a n", p=P))
            wv_sb = singles.tile([P, 8, DHL], BF16)
            nc.sync.dma_start(wv_sb, wv_d.rearrange("(a p) n -> p a n", p=P))
            wo_sb = singles.tile([P, 2, D], BF16)
            nc.sync.dma_start(wo_sb, wo_d.rearrange("(a p) n -> p a n", p=P))

            # ---------- K^T projection ----------
            KT = singles.tile([P, 2, nkb * P], BF16)
            for sc in range(n_ksc):
                kTc = xtp.tile([P, 8, 512], BF16, tag="xT")
                nc.sync.dma_start_transpose(kTc, k_bf[sc * 512:(sc + 1) * 512, :])
                w = min(512, nkb * P - sc * 512)
                for ob in range(2):
                    ps = psMM.tile([P, 512], F32, tag="mm")
                    for ib in range(8):
                        nc.tensor.matmul(ps[:, :w], wk_sb[:, ib, ob * P:(ob + 1) * P],
                                         kTc[:, ib, :w], start=(ib == 0), stop=(ib == 7))
                    nc.vector.tensor_copy(KT[:, ob, sc * 512:sc * 512 + w], ps[:, :w])

            # ---------- V projection + mask + denom column ----------
            Vp = singles.tile([P, nkb, HL * 65], BF16)
            for sc in range(n_ksc):
                vTc = xtp.tile([P, 8, 512], BF16, tag="xT")
                nc.sync.dma_start_transpose(vTc, v_bf[sc * 512:(sc + 1) * 512, :])
                for s4 in range(4):
                    kb = sc * 4 + s4
                    if kb >= nkb:
                        break
                    ps = psMM.tile([P, 512], F32, tag="mm")
                    for ib in range(8):
                        nc.tensor.matmul(ps[:, :DHL], vTc[:, ib, s4 * P:(s4 + 1) * P],
                                         wv_sb[:, ib, :], start=(ib == 0), stop=(ib == 7))
                    vsl = Vp[:, kb, :].rearrange("p (h e) -> p h e", h=HL)
                    m1 = mask_sb[:, kb:kb + 1, None]
                    nc.vector.tensor_tensor(
                        vsl[:, :, 0:64],
                        ps[:, :DHL].rearrange("p (h e) -> p h e", e=64),
                        m1.to_broadcast([P, HL, 64]), OP.mult)
                    nc.vector.tensor_copy(vsl[:, :, 64:65],
                                          m1.to_broadcast([P, HL, 1]))

            # big/late-needed weight + param loads AFTER the K/V projection DMAs
            # so they don't congest the queues at kernel start
            b2rep = singles.tile([P, D], BF16)
            nc.sync.dma_start(b2rep, bcast(b2_d, P))
            lnp_sb = singles.tile([P, 4, D], BF16)
            nc.sync.dma_start(lnp_sb, bcast(lnp_d, P))
            w2_sb = singles.tile([P, FF // P, D], BF16)
            w2_src = w2_d.rearrange("(a p) n -> p a n", p=P)
            for wc in range(4):
                nc.sync.dma_start(w2_sb[:, wc * 8:(wc + 1) * 8, :],
                                  w2_src[:, wc * 8:(wc + 1) * 8, :])

            # ---------- helpers for FFN phase ----------
            def ffn_chunk(ch, rs_out):
                ybfs = []
                yT = ytp.tile([P, 8, 256], BF16, tag="yT")
                for sblk in range(2):
                    xsb = ffp.tile([P, D], BF16, tag="rsx")
                    nc.sync.dma_start(
                        xsb, rs_out.rearrange("(t p) d -> p t d", p=P)[:, sblk, :])
                    qrs = ffp.tile([P, D], BF16, tag="qrs")
                    nc.sync.dma_start(
                        qrs, qres_d[ch * 256 + sblk * P: ch * 256 + (sblk + 1) * P, :])
                    xx = ffp.tile([P, D], BF16, tag="xx")
                    nc.vector.tensor_tensor(xx, xsb, qrs, OP.add)
                    ybf = ffp.tile([P, D], BF16, tag="ybf")
                    _layernorm(xx, 0, ybf, F32_out=None)
                    ybfs.append(ybf)
                    for db in range(8):
                        tp = psMM.tile([P, P], BF16, tag="mm")
                        nc.tensor.transpose(tp, ybf[:, db * P:(db + 1) * P], ident)
                        nc.vector.tensor_copy(yT[:, db, sblk * P:(sblk + 1) * P], tp)
                h1T = h1p.tile([P, FF // P, 256], BF16, tag="h1T")
                for fb in range(FF // P):
                    w1t = w1p.tile([P, 8, P], BF16, tag="w1t")
                    nc.sync.dma_start(
                        w1t, w1_d.rearrange("(a p) f -> p a f", p=P)[:, :, fb * P:(fb + 1) * P])
                    hps = psMM.tile([P, 256], F32, tag="mm")
                    for db in range(8):
                        nc.tensor.matmul(hps, w1t[:, db, :], yT[:, db, :],
                                         start=(db == 0), stop=(db == 7))
                    nc.scalar.activation(h1T[:, fb, :], hps, AF.Relu,
                                         bias=b1_sb[:, fb:fb + 1])
                for sblk in range(2):
                    fy = ffp.tile([P, D], BF16, tag="xx")
                    for dc in range(2):
                        fps = psMM.tile([P, 512], F32, tag="mm")
                        for fb in range(FF // P):
                            nc.tensor.matmul(fps, h1T[:, fb, sblk * P:(sblk + 1) * P],
                                             w2_sb[:, fb, dc * 512:(dc + 1) * 512],
                                             start=(fb == 0), stop=(fb == FF // P - 1))
                        nc.vector.tensor_tensor(fy[:, dc * 512:(dc + 1) * 512], fps,
                                                b2rep[:, dc * 512:(dc + 1) * 512], OP.add)
                    nc.vector.tensor_tensor(fy, fy, ybfs[sblk], OP.add)
                    ost = ffp.tile([P, D], F32, tag="ost")
                    _layernorm(fy, 2, None, F32_out=ost)
                    nc.sync.dma_start(
                        out_d[ch * 256 + sblk * P: ch * 256 + (sblk + 1) * P, :], ost)

            def _layernorm(xx, gidx, bf16_out, F32_out):
                """LN over free dim D of xx [P, D]; gain=lnp[gidx], bias=lnp[gidx+1]."""
                stats = smallp.tile([P, 2, 6], F32, tag="stats")
                for h in range(2):
                    nc.vector.bn_stats(stats[:, h, :], xx[:, h * 512:(h + 1) * 512])
                mv = smallp.tile([P, 2], F32, tag="mv")
                nc.vector.bn_aggr(mv, stats)
                lnv = smallp.tile([P, 1], F32, tag="lnv")
                nc.scalar.activation(lnv, mv[:, 1:2], AF.Ln, bias=eps_sb)
                rstd = smallp.tile([P, 1], F32, tag="rstd")
                nc.scalar.activation(rstd, lnv, AF.Exp, scale=-0.5)
                t = lntp.tile([P, D], F32, tag="lnt")
                nc.vector.tensor_scalar(t, xx, mv[:, 0:1], rstd,
                                        OP.subtract, OP.mult)
                g = lnp_sb[:, gidx, :]
                bvec = lnp_sb[:, gidx + 1, :]
                if bf16_out is not None:
                    nc.vector.tensor_tensor(t, t, g, OP.mult)
                    nc.vector.tensor_tensor(bf16_out, t, bvec, OP.add)
                else:
                    nc.vector.tensor_tensor(t, t, g, OP.mult)
                    nc.vector.tensor_tensor(F32_out, t, bvec, OP.add)

            # ---------- main attention loop ----------
            cin = [None, None]
            rs_out = [None, None]
            for ch in range(NCH):
                cin[ch] = dramp.tile([1024, D], BF16, tag="cin", name=f"cin{ch}")
                rs_out[ch] = dramp.tile([256, D], BF16, tag="rsout", name=f"rsout{ch}")

            for qc in range(NQC):
                ch = qc // 2
                # Q^T projection for this 512-query chunk
                qTc = xtp.tile([P, 8, 512], BF16, tag="xT")
                nc.sync.dma_start_transpose(qTc, q_bf[qc * 512:(qc + 1) * 512, :])
                QT = qtp.tile([P, 2, 512], BF16, tag="QT")
                for ob in range(2):
                    ps = psMM.tile([P, 512], F32, tag="mm")
                    for ib in range(8):
                        nc.tensor.matmul(ps, wq_sb[:, ib, ob * P:(ob + 1) * P],
                                         qTc[:, ib, :], start=(ib == 0), stop=(ib == 7))
                    nc.vector.tensor_copy(QT[:, ob, :], ps)

                ctxT_sb = ctp.tile([P, 2, 512], BF16, tag="ctxT")
                for hp in range(2):
                    ctxA = psCtx.tile([65, 512], F32, tag="ctx")
                    ctxB = psCtx.tile([65, 512], F32, tag="ctx")
                    for kb in range(nkb):
                        st = psS.tile([P, 1024], F32, tag="st")
                        nc.tensor.matmul(st[:, 0:512],
                                         KT[0:64, hp, kb * P:(kb + 1) * P],
                                         QT[0:64, hp, :],
                                         tile_position=(0, 0), start=True, stop=True)
                        nc.tensor.matmul(st[:, 512:1024],
                                         KT[64:128, hp, kb * P:(kb + 1) * P],
                                         QT[64:128, hp, :],
                                         tile_position=(64, 0), start=True, stop=True)
                        Pt = ptp.tile([P, 1024], BF16, tag="Pt")
                        nc.scalar.activation(Pt, st, AF.Exp, scale=0.125)
                        vsl = Vp[:, kb, :].rearrange("p (h e) -> p h e", h=HL)
                        nc.tensor.matmul(ctxA, vsl[:, 2 * hp, :], Pt[:, 0:512],
                                         start=(kb == 0), stop=(kb == nkb - 1))
                        nc.tensor.matmul(ctxB, vsl[:, 2 * hp + 1, :], Pt[:, 512:1024],
                                         start=(kb == 0), stop=(kb == nkb - 1))
                    # divide by denominators (row 64), assemble ctxT block hp
                    rA = recipp.tile([1, 512], F32, tag="recip")
                    nc.vector.reciprocal(rA, ctxA[64:65, :])
                    rB = recipp.tile([1, 512], F32, tag="recip")
                    nc.vector.reciprocal(rB, ctxB[64:65, :])
                    # partition-broadcast via DRAM bounce (SBUF src can't step-0)
                    # on gpsimd SWDGE so these latency-critical small DMAs skip
                    # the bulk sync-queue traffic
                    rd = dramp.tile([2, 512], F32, tag="rd", name=f"rd_{qc}_{hp}")
                    nc.gpsimd.dma_start(rd[0:1, :], rA)
                    nc.gpsimd.dma_start(rd[1:2, :], rB)
                    rbA = rbp.tile([64, 512], F32, tag="rb")
                    nc.gpsimd.dma_start(rbA, bcast(rd[0, :], 64))
                    rbB = rbp.tile([64, 512], F32, tag="rb")
                    nc.gpsimd.dma_start(rbB, bcast(rd[1, :], 64))
                    nc.vector.tensor_tensor(ctxT_sb[0:64, hp, :], ctxA[0:64, :],
                                            rbA, OP.mult)
                    tmpB = tmbp.tile([64, 512], BF16, tag="tmpB")
                    nc.vector.tensor_tensor(tmpB, ctxB[0:64, :], rbB, OP.mult)
                    nc.gpsimd.dma_start(ctxT_sb[64:128, hp, :], tmpB)

                # Wo partial for this chunk -> collective input
                for sblk in range(4):
                    ao = aop.tile([P, D], BF16, tag="ao")
                    for dc in range(2):
                        ps = psMM.tile([P, 512], F32, tag="mm")
                        for db in range(2):
                            nc.tensor.matmul(ps, ctxT_sb[:, db, sblk * P:(sblk + 1) * P],
                                             wo_sb[:, db, dc * 512:(dc + 1) * 512],
                                             start=(db == 0), stop=(db == 1))
                        nc.vector.tensor_copy(ao[:, dc * 512:(dc + 1) * 512], ps)
                    nc.sync.dma_start(
                        cin[ch].rearrange("(t p) d -> p t d", p=P)[:, (qc % 2) * 4 + sblk, :],
                        ao)

                if qc == 1:
                    nc.gpsimd.collective_compute(
                        "ReduceScatter", OP.add,
                        replica_groups=[[0, 1, 2, 3], [4, 5, 6, 7]],
                        ins=[cin[0].opt()], outs=[rs_out[0].opt()])
                if qc == 2:
                    ffn_chunk(0, rs_out[0])
                if qc == 3:
                    nc.gpsimd.collective_compute(
                        "ReduceScatter", OP.add,
                        replica_groups=[[0, 1, 2, 3], [4, 5, 6, 7]],
                        ins=[cin[1].opt()], outs=[rs_out[1].opt()])
                    ffn_chunk(1, rs_out[1])

    nc.compile()
    return nc


_CACHE = {}


def _get_nc(nkb):
    if nkb not in _CACHE:
        _CACHE[nkb] = _build(nkb)
    return _CACHE[nkb]


LAST_RESULT = None
LAST_CTX = None


def kernel(q, k, v, Wq, Wk, Wv, Wo, w1, b1, w2, b2,
           ln1_g, ln1_b, ln2_g, ln2_b, valid_lens, _trace=False):
    global LAST_RESULT
    bf = ml_dtypes.bfloat16
    q = np.asarray(q, np.float32); k = np.asarray(k, np.float32)
    v = np.asarray(v, np.float32)
    vl = np.asarray(valid_lens).astype(np.int64)
    nkb = int(min(S // P, max(1, math.ceil(float(vl.max()) / P))))
    nc = _get_nc(nkb)

    w1b = np.ascontiguousarray(np.asarray(w1, np.float32)).astype(bf)
    w2b = np.ascontiguousarray(np.asarray(w2, np.float32)).astype(bf)
    lnp = np.stack([np.asarray(x, np.float32) for x in (ln1_g, ln1_b, ln2_g, ln2_b)]
                   ).astype(bf)
    b1f = np.asarray(b1, np.float32)
    b2b = np.asarray(b2, np.float32).astype(bf)

    in_maps = []
    tok_idx_all = []
    for c in range(8):
        b = c // 4
        r = c % 4
        cols = slice(r * DHL, (r + 1) * DHL)
        mask = (np.arange(S) < int(vl[b])).astype(np.float32)
        tok_idx = np.concatenate(
            [ch * 1024 + r * 256 + np.arange(256) for ch in range(NCH)])
        tok_idx_all.append(tok_idx)
        in_maps.append({
            "q_bf": q[b].astype(bf),
            "k_bf": k[b].astype(bf),
            "v_bf": v[b].astype(bf),
            "wq": np.ascontiguousarray(np.asarray(Wq, np.float32)[:, cols]).astype(bf),
            "wk": np.ascontiguousarray(np.asarray(Wk, np.float32)[:, cols]).astype(bf),
            "wv": np.ascontiguousarray(np.asarray(Wv, np.float32)[:, cols]).astype(bf),
            "wo": np.ascontiguousarray(np.asarray(Wo, np.float32)[cols, :]).astype(bf),
            "w1": w1b, "w2": w2b, "b1f": b1f, "b2b": b2b, "lnp": lnp,
            "maskf": mask,
            "qres": np.ascontiguousarray(q[b][tok_idx]).astype(bf),
        })

    res = bass_utils.run_bass_kernel_spmd(nc, in_maps, core_ids=list(range(8)),
                                          trace=_trace)
    LAST_RESULT = res
    global LAST_CTX
    LAST_CTX = (nc, in_maps, nkb)

    out = np.empty((B, S, D), np.float32)
    for c in range(8):
        out[c // 4, tok_idx_all[c]] = res.results[c]["out"]
    return out
